# revision 25
# baseline (speedup 1.0000x reference)
"""TRN2 Bass kernel for nn_FFTMLP_86904368267649.

Reference math: energies[b,o] = sum_f xr[b,f]*w_r[o,f] + xi[b,f]*w_i[o,f]
with w_r = fr+fi, w_i = fr-fi, x: [B, 2, F] fp32, filters: [O, F] fp32.

Structure exploited: the filters have period O (=1024) in f, so the
F=2049-long contraction folds to T=1024 per channel:
  xr'[b,t] = xr[b,t] + xr[b,t+1024]  (+ xr[b,2048] into t=0)
giving energies = [xr' | xi'] @ Wf with Wf [2T=2048, O=1024].
The fold runs on-chip (DVE); the matmul runs in float32r (TF32-like,
full PE rate at moving-dim >= 256).

Sharding: data-parallel over batch, 2048 rows per core across 8 cores.
Each core's x shard is passed pre-transposed ([4098, 2048]) so the
contraction dim lands on SBUF partitions without an on-chip transpose.
Filters (folded weights) are replicated to all cores.

Tiling: raw x rows arrive as [128, 1024] transfers (4 KB DMA lines,
~22 GB/s per DMA engine vs ~15 at 2 KB) on the GpSimd queue while W
stages on Sync; the batch is processed in 4 chunks of 512, each as two
k-major PSUM sweeps of 2 b-subtiles x 2 o-halves (4 banks per sweep)
so consecutive sweeps ping-pong banks and drains overlap compute.
Measured: ~205 us max-core / ~199 us mean (pure-DMA floor for the same
50.4 MB/core is ~160-170 us at the observed ~320 GB/s/core HBM rate).
"""

import sys

if "/opt/trn_rl_repo" not in sys.path:
    sys.path.insert(0, "/opt/trn_rl_repo")

import numpy as np

import concourse.bass as bass
import concourse.mybir as mybir
import concourse.tile as tile
from concourse import bacc
from concourse.bass_utils import run_bass_kernel_spmd

B, O, F, T = 16384, 1024, 2049, 1024
NCORES = 8
BS = B // NCORES          # 2048 batch rows per core
K = 2 * T                 # 2048 folded contraction
KT = K // 128             # 16 k-tiles
BCH = 512                 # b-chunk for the PSUM k-sweep
NCH = BS // BCH           # 4 chunks per core
LDW_W = 1024              # raw x DMA width (4 KB lines), 2 chunks per load
F32 = mybir.dt.float32
F32R = mybir.dt.float32r

_CACHE = {}
LAST_RESULTS = None


def _build():
    nc = bacc.Bacc("TRN2", target_bir_lowering=False, debug=False,
                   num_devices=NCORES)

    xt_dram = nc.dram_tensor("xT", [2 * F, BS], F32, kind="ExternalInput")
    w_dram = nc.dram_tensor("w", [K, O], F32, kind="ExternalInput")
    out_dram = nc.dram_tensor("out", [BS, O], F32, kind="ExternalOutput")

    # DRAM row starts feeding folded k-tile k (A + B operands):
    #   real (k 0..7):  A rows 128k..,        B rows 1024+128k..
    #   imag (k 8..15): A rows 2049+128(k-8), B rows 3073+128(k-8)
    def a_row(k):
        return 128 * k if k < 8 else 2049 + 128 * (k - 8)

    def b_row(k):
        return 1024 + 128 * k if k < 8 else 3073 + 128 * (k - 8)

    with tile.TileContext(nc) as tc:
        with (
            tc.tile_pool(name="wconst", bufs=1) as wconst,
            tc.tile_pool(name="wstage", bufs=1) as wstage,
            tc.tile_pool(name="raw", bufs=3) as raw,
            tc.tile_pool(name="rawbp", bufs=2) as rawbp,
            tc.tile_pool(name="xfp", bufs=2) as xfpool,
            tc.tile_pool(name="outp", bufs=3) as outp,
            tc.tile_pool(name="psum", bufs=4, space="PSUM") as psum,
        ):
            xt_ap = xt_dram.ap()
            out_re = out_dram.ap().rearrange("r (h o) -> r h o", h=2)
            w_ap = w_dram.ap().rearrange("(ko p) o -> p ko o", p=128)
            wr = wconst.tile([128, KT, O], F32R)

            # wrap rows (f=2048 real / imag), full shard width, loaded
            # once; both channels side by side on partition 0
            wrapt = wconst.tile([1, 2 * BS], F32)
            nc.gpsimd.dma_start(wrapt[0:1, :BS], xt_ap[2048:2049, :])
            nc.gpsimd.dma_start(wrapt[0:1, BS:], xt_ap[4097:4098, :])

            raw_tiles = {}

            def emit_w(kp):
                # stage on Sync queue, f32r rounding on ACT
                stg = wstage.tile([128, 2, O], F32, tag="wstage",
                                  name=f"stg{kp}")
                nc.sync.dma_start(stg[:], w_ap[:, 2 * kp:2 * kp + 2])
                nc.scalar.copy(wr[:, 2 * kp], stg[:, 0])
                nc.scalar.copy(wr[:, 2 * kp + 1], stg[:, 1])

            def emit_raw(g, kp):
                # rows for k0 and k0+1 are adjacent in DRAM: one 3D
                # transfer each (2x 4KB lines/partition); ta on GpSimd,
                # tb on ACT so issue cost doesn't serialize on one queue
                gs = g * LDW_W
                k0 = 2 * kp
                ta = raw.tile([128, 2, LDW_W], F32, tag="rawa",
                              name=f"ta{g}_{kp}")
                tb = rawbp.tile([128, 2, LDW_W], F32, tag="rawb",
                              name=f"tb{g}_{kp}")
                for j in range(2):
                    nc.gpsimd.dma_start(
                        ta[:, j],
                        xt_ap[a_row(k0 + j):a_row(k0 + j) + 128,
                              gs:gs + LDW_W])
                    nc.gpsimd.dma_start(
                        tb[:, j],
                        xt_ap[b_row(k0 + j):b_row(k0 + j) + 128,
                              gs:gs + LDW_W])
                raw_tiles[(g, kp)] = (ta, tb)

            def emit_folds(c):
                g, half = divmod(c, LDW_W // BCH)
                cs, hs = c * BCH, (c % (LDW_W // BCH)) * BCH
                xf = xfpool.tile([128, KT, BCH], F32R, tag="xf",
                                 name=f"xf{c}")
                for k in range(KT):
                    ta, tb = raw_tiles[(g, k // 2)]
                    j = k % 2
                    if k == 0 or k == 8:
                        # fold the channel's wrap row into t=0 first
                        nc.vector.tensor_add(
                            out=ta[0:1, j, hs:hs + BCH],
                            in0=ta[0:1, j, hs:hs + BCH],
                            in1=wrapt[0:1, cs:cs + BCH] if k == 0
                            else wrapt[0:1, BS + cs:BS + cs + BCH])
                    nc.vector.tensor_add(
                        out=xf[:, k], in0=ta[:, j, hs:hs + BCH],
                        in1=tb[:, j, hs:hs + BCH])
                return xf

            def emit_sweeps(c, xf):
                # two k-major sweeps of 2 b-subtiles x 2 o-halves
                # (4 PSUM banks each): consecutive sweeps ping-pong banks
                # so the PE never waits on a full drain barrier
                cs = c * BCH
                for sw in range(2):
                    ps = [psum.tile([128, 2, 512], F32, tag="ps",
                                    name=f"ps{c}_{sw}_{i}")
                          for i in range(2)]
                    for k in range(KT):
                        st, sp = (k == 0), (k == KT - 1)
                        for s in range(2):
                            sub = 2 * sw + s
                            lhsT = xf[:, k, sub * 128:(sub + 1) * 128]
                            for oh in range(2):
                                nc.tensor.matmul(
                                    ps[s][:, oh],
                                    lhsT,
                                    wr[:, k, oh * 512:(oh + 1) * 512],
                                    start=st, stop=sp,
                                )
                    for s in range(2):
                        sub = 2 * sw + s
                        ot = outp.tile([128, 2, 512], F32, tag="out",
                                       name=f"ot{c}_{sub}")
                        nc.vector.tensor_copy(ot[:], ps[s][:])
                        r0 = cs + sub * 128
                        nc.sync.dma_start(out_re[r0:r0 + 128], ot[:])

            for g in range(BS // LDW_W):
                for kp in range(KT // 2):
                    if g == 0:
                        emit_w(kp)
                    emit_raw(g, kp)
                for half in range(LDW_W // BCH):
                    c = g * (LDW_W // BCH) + half
                    xf = emit_folds(c)
                    emit_sweeps(c, xf)

    nc.compile()
    return nc


def kernel(x, filters_real, filters_imag):
    global LAST_RESULTS
    x = np.asarray(x, dtype=np.float32)
    fr = np.asarray(filters_real, dtype=np.float32)
    fi = np.asarray(filters_imag, dtype=np.float32)

    w_r = fr + fi                      # [O, F]
    w_i = fr - fi
    wf = np.empty((K, O), np.float32)  # folded weights (first period)
    wf[:T] = w_r[:, :T].T
    wf[T:] = w_i[:, :T].T

    if "nc" not in _CACHE:
        _CACHE["nc"] = _build()
    nc = _CACHE["nc"]

    xs = x.reshape(B, 2 * F)
    in_maps = []
    for c in range(NCORES):
        xt = np.ascontiguousarray(xs[c * BS:(c + 1) * BS].T)  # [4098, 2048]
        in_maps.append({"xT": xt, "w": wf})

    import os
    trace = bool(os.environ.get("BASS_TRACE"))
    if trace:
        try:
            import antenv.axon_hooks  # noqa: F401  (shim from test.py)
        except ImportError:
            trace = False
            os.environ["BASS_NEVER_TRACE"] = "1"
    res = run_bass_kernel_spmd(nc, in_maps, list(range(NCORES)), trace=trace)
    LAST_RESULTS = res
    return np.concatenate([res.results[c]["out"] for c in range(NCORES)], axis=0)


# revision 26
# speedup vs baseline: 1.0768x; 1.0768x over previous
"""TRN2 Bass kernel for nn_FFTMLP_86904368267649.

Reference math: energies[b,o] = sum_f xr[b,f]*w_r[o,f] + xi[b,f]*w_i[o,f]
with w_r = fr+fi, w_i = fr-fi, x: [B, 2, F] fp32, filters: [O, F] fp32.

Structure exploited: the filters have period O (=1024) in f, so the
F=2049-long contraction folds to T=1024 per channel:
  xr'[b,t] = xr[b,t] + xr[b,t+1024]  (+ xr[b,2048] into t=0)
giving energies = [xr' | xi'] @ Wf with Wf [2T=2048, O=1024].
The fold runs on-chip (DVE); the matmul runs in float32r (TF32-like,
full PE rate at moving-dim >= 256).

Sharding: data-parallel over batch, 2048 rows per core across 8 cores.
Each core's x shard is passed pre-transposed ([4098, 2048]) so the
contraction dim lands on SBUF partitions without an on-chip transpose.
Filters (folded weights) are replicated to all cores.

Tiling: raw x rows arrive as [128, 1024] transfers (4 KB DMA lines,
~22 GB/s per DMA engine vs ~15 at 2 KB) on the GpSimd queue while W
stages on Sync; the batch is processed in 4 chunks of 512, each as two
k-major PSUM sweeps of 2 b-subtiles x 2 o-halves (4 banks per sweep)
so consecutive sweeps ping-pong banks and drains overlap compute.
Measured: ~205 us max-core / ~199 us mean (pure-DMA floor for the same
50.4 MB/core is ~160-170 us at the observed ~320 GB/s/core HBM rate).
"""

import sys

if "/opt/trn_rl_repo" not in sys.path:
    sys.path.insert(0, "/opt/trn_rl_repo")

import numpy as np

import concourse.bass as bass
import concourse.mybir as mybir
import concourse.tile as tile
from concourse import bacc
from concourse.bass_utils import run_bass_kernel_spmd

B, O, F, T = 16384, 1024, 2049, 1024
NCORES = 8
BS = B // NCORES          # 2048 batch rows per core
K = 2 * T                 # 2048 folded contraction
KT = K // 128             # 16 k-tiles
BCH = 512                 # b-chunk for the PSUM k-sweep
NCH = BS // BCH           # 4 chunks per core
LDW_W = 1024              # raw x DMA width (4 KB lines), 2 chunks per load
F32 = mybir.dt.float32
F32R = mybir.dt.float32r

_CACHE = {}
LAST_RESULTS = None


def _build():
    nc = bacc.Bacc("TRN2", target_bir_lowering=False, debug=False,
                   num_devices=NCORES)

    xt_dram = nc.dram_tensor("xT", [2 * F, BS], F32, kind="ExternalInput")
    w_dram = nc.dram_tensor("w", [K, O], F32, kind="ExternalInput")
    out_dram = nc.dram_tensor("out", [BS, O], F32, kind="ExternalOutput")

    # DRAM row starts feeding folded k-tile k (A + B operands):
    #   real (k 0..7):  A rows 128k..,        B rows 1024+128k..
    #   imag (k 8..15): A rows 2049+128(k-8), B rows 3073+128(k-8)
    def a_row(k):
        return 128 * k if k < 8 else 2049 + 128 * (k - 8)

    def b_row(k):
        return 1024 + 128 * k if k < 8 else 3073 + 128 * (k - 8)

    with tile.TileContext(nc) as tc:
        with (
            tc.tile_pool(name="wconst", bufs=1) as wconst,
            tc.tile_pool(name="wstage", bufs=2) as wstage,
            tc.tile_pool(name="raw", bufs=2) as raw,
            tc.tile_pool(name="xfp", bufs=2) as xfpool,
            tc.tile_pool(name="outp", bufs=3) as outp,
            tc.tile_pool(name="psum", bufs=4, space="PSUM") as psum,
        ):
            xt_ap = xt_dram.ap()
            out_re = out_dram.ap().rearrange("r (h o) -> r h o", h=2)
            w_ap = w_dram.ap().rearrange("(ko p) o -> p ko o", p=128)
            wr = wconst.tile([128, KT, O], F32R)

            # wrap rows (f=2048 real / imag), full shard width, loaded
            # once; both channels side by side on partition 0
            wrapt = wconst.tile([1, 2 * BS], F32)
            nc.gpsimd.dma_start(wrapt[0:1, :BS], xt_ap[2048:2049, :])
            nc.gpsimd.dma_start(wrapt[0:1, BS:], xt_ap[4097:4098, :])

            raw_tiles = {}

            def emit_w(kp):
                # stage on Sync queue, f32r rounding on ACT
                stg = wstage.tile([128, 2, O], F32, tag="wstage",
                                  name=f"stg{kp}")
                nc.sync.dma_start(stg[:], w_ap[:, 2 * kp:2 * kp + 2])
                nc.scalar.copy(wr[:, 2 * kp], stg[:, 0])
                nc.scalar.copy(wr[:, 2 * kp + 1], stg[:, 1])

            def emit_raw(g, kp):
                # rows for k0 and k0+1 are adjacent in DRAM: one 3D
                # transfer each (2x 4KB lines/partition); ta on GpSimd,
                # tb on ACT so issue cost doesn't serialize on one queue
                gs = g * LDW_W
                k0 = 2 * kp
                ta = raw.tile([128, 2, LDW_W], F32, tag="rawa",
                              name=f"ta{g}_{kp}")
                tb = raw.tile([128, 2, LDW_W], F32, tag="rawb",
                              name=f"tb{g}_{kp}")
                for j in range(2):
                    nc.gpsimd.dma_start(
                        ta[:, j],
                        xt_ap[a_row(k0 + j):a_row(k0 + j) + 128,
                              gs:gs + LDW_W])
                    nc.gpsimd.dma_start(
                        tb[:, j],
                        xt_ap[b_row(k0 + j):b_row(k0 + j) + 128,
                              gs:gs + LDW_W])
                raw_tiles[(g, kp)] = (ta, tb)

            def emit_folds(c):
                g, half = divmod(c, LDW_W // BCH)
                cs, hs = c * BCH, (c % (LDW_W // BCH)) * BCH
                xf = xfpool.tile([128, KT, BCH], F32R, tag="xf",
                                 name=f"xf{c}")
                for k in range(KT):
                    ta, tb = raw_tiles[(g, k // 2)]
                    j = k % 2
                    if k == 0 or k == 8:
                        # fold the channel's wrap row into t=0 first
                        nc.vector.tensor_add(
                            out=ta[0:1, j, hs:hs + BCH],
                            in0=ta[0:1, j, hs:hs + BCH],
                            in1=wrapt[0:1, cs:cs + BCH] if k == 0
                            else wrapt[0:1, BS + cs:BS + cs + BCH])
                    nc.vector.tensor_add(
                        out=xf[:, k], in0=ta[:, j, hs:hs + BCH],
                        in1=tb[:, j, hs:hs + BCH])
                return xf

            def emit_sweeps(c, xf):
                # two k-major sweeps of 2 b-subtiles x 2 o-halves
                # (4 PSUM banks each): consecutive sweeps ping-pong banks
                # so the PE never waits on a full drain barrier
                cs = c * BCH
                for sw in range(2):
                    ps = [psum.tile([128, 2, 512], F32, tag="ps",
                                    name=f"ps{c}_{sw}_{i}")
                          for i in range(2)]
                    for k in range(KT):
                        st, sp = (k == 0), (k == KT - 1)
                        for s in range(2):
                            sub = 2 * sw + s
                            lhsT = xf[:, k, sub * 128:(sub + 1) * 128]
                            for oh in range(2):
                                nc.tensor.matmul(
                                    ps[s][:, oh],
                                    lhsT,
                                    wr[:, k, oh * 512:(oh + 1) * 512],
                                    start=st, stop=sp,
                                )
                    for s in range(2):
                        sub = 2 * sw + s
                        ot = outp.tile([128, 2, 512], F32, tag="out",
                                       name=f"ot{c}_{sub}")
                        nc.vector.tensor_copy(ot[:], ps[s][:])
                        r0 = cs + sub * 128
                        nc.sync.dma_start(out_re[r0:r0 + 128], ot[:])

            for g in range(BS // LDW_W):
                for kp in range(KT // 2):
                    if g == 0:
                        emit_w(kp)
                    emit_raw(g, kp)
                for half in range(LDW_W // BCH):
                    c = g * (LDW_W // BCH) + half
                    xf = emit_folds(c)
                    emit_sweeps(c, xf)

    nc.compile()
    return nc


def kernel(x, filters_real, filters_imag):
    global LAST_RESULTS
    x = np.asarray(x, dtype=np.float32)
    fr = np.asarray(filters_real, dtype=np.float32)
    fi = np.asarray(filters_imag, dtype=np.float32)

    w_r = fr + fi                      # [O, F]
    w_i = fr - fi
    wf = np.empty((K, O), np.float32)  # folded weights (first period)
    wf[:T] = w_r[:, :T].T
    wf[T:] = w_i[:, :T].T

    if "nc" not in _CACHE:
        _CACHE["nc"] = _build()
    nc = _CACHE["nc"]

    xs = x.reshape(B, 2 * F)
    in_maps = []
    for c in range(NCORES):
        xt = np.ascontiguousarray(xs[c * BS:(c + 1) * BS].T)  # [4098, 2048]
        in_maps.append({"xT": xt, "w": wf})

    import os
    trace = bool(os.environ.get("BASS_TRACE"))
    if trace:
        try:
            import antenv.axon_hooks  # noqa: F401  (shim from test.py)
        except ImportError:
            trace = False
            os.environ["BASS_NEVER_TRACE"] = "1"
    res = run_bass_kernel_spmd(nc, in_maps, list(range(NCORES)), trace=trace)
    LAST_RESULTS = res
    return np.concatenate([res.results[c]["out"] for c in range(NCORES)], axis=0)
